# revision 11
# baseline (speedup 1.0000x reference)
# Trainium2 Bass kernel for nn_AxonalConnections (gnn_message_passing).
#
# Computes out[B, H, W] = (spikes.reshape(B, N) @ adjacency.T).reshape(B, H, W)
# with B=16, H=W=128, N=16384 on 8 NeuronCores.
#
# Strategy (pure tensor parallelism, no collectives):
#   - Shard adjacency row-wise (target dim) across 8 cores: core i owns
#     target columns [i*2048, (i+1)*2048) of the output.
#   - Host-side, transpose each shard to [source, target] layout so the
#     contraction dim (source) lands on SBUF partitions with unit-stride DMAs.
#   - The kernel is HBM-bandwidth bound, so minimize bytes: adjacency is
#     shipped as fp16 (values are ~N(0, 0.02^2), well inside fp16 range;
#     2^-11 relative representation error -> ~1e-4 output error). fp32
#     matmul would also stream 4x slower through the PE; fp16 streams at
#     full rate (1 column/cycle).
#   - Spikes (tiny) are split into fp16 hi + fp16 lo (exact to ~2^-22) and
#     packed as the stationary operand [spikes_hi | spikes_lo] (32 columns).
#     PSUM accumulates [32, 2048] fp32; rows 0-15 = hi terms, rows 16-31 =
#     lo terms; host folds them and concatenates the target shards.
#
# Per-core traffic: 64 MiB adjacency + 1 MiB spikes; single-queue HWDGE DMA
# sustains ~420 GB/s -> ~155 us steady state + ~25 us head/tail.

import numpy as np

B = 16
H = 128
W = 128
N = H * W            # 16384 source == target size
NCORES = 8
TSH = N // NCORES    # 2048 target columns per core
P = 128              # SBUF partitions / contraction tile
SCHUNKS = N // P     # 128 source chunks
GROUP = 4            # source chunks per DMA (GROUP * 0.5 MiB per transfer)
NFREE = 512          # matmul moving free dim (one PSUM bank of fp32)

_cache = {}


def _build_nc():
    import concourse.mybir as mybir
    import concourse.tile as tile
    from concourse import bacc

    nc = bacc.Bacc(
        "TRN2",
        target_bir_lowering=False,
        debug=False,
        num_devices=NCORES,
    )
    # a16: adjacency shard, transposed to [source, target] fp16, with GROUP
    # source-chunks packed per DRAM row so every DMA descriptor moves one
    # maximal contiguous run per partition (short runs throttle DMA: the
    # SDMA per-packet overhead is ~12-20 ns regardless of size).
    #   a16[g*128 + p, nl*TSH + t] = fp16(adj[t0 + t, (GROUP*g + nl)*128 + p])
    a16 = nc.dram_tensor(
        "a16", [N // GROUP, GROUP * TSH], mybir.dt.float16, kind="ExternalInput"
    ).ap()
    # spk: stationary weights, packed [P, SCHUNKS*32] fp16 where
    #   spk[p, n*32 + b]      = fp16_hi(spikes[b, n*128 + p])
    #   spk[p, n*32 + 16 + b] = fp16_lo(spikes[b, n*128 + p])
    spk = nc.dram_tensor(
        "spk", [P, SCHUNKS * 32], mybir.dt.float16, kind="ExternalInput"
    ).ap()
    out = nc.dram_tensor("o", [32, TSH], mybir.dt.float32, kind="ExternalOutput").ap()

    f32 = mybir.dt.float32
    f16 = mybir.dt.float16
    NJ = TSH // NFREE  # 4 PSUM banks

    with tile.TileContext(nc) as tc:
        with (
            tc.tile_pool(name="adj", bufs=5) as adj_pool,
            tc.tile_pool(name="spkp", bufs=1) as spk_pool,
            tc.tile_pool(name="psum", bufs=1, space="PSUM") as psum_pool,
            tc.tile_pool(name="outp", bufs=1) as out_pool,
        ):
            # Stationary weights go on the ACT HWDGE ring so the transfer
            # overlaps the first adjacency groups on the SP ring (the
            # two-queue engine-utilization penalty only matters in steady
            # state; here the ACT ring drains in the first few us). The
            # adjacency stream itself stays on ONE queue: splitting it
            # across SP and ACT makes each SDMA engine alternate queues per
            # packet, costing ~15% utilization (measured 320 vs 422 GB/s).
            spk_t = spk_pool.tile([P, SCHUNKS * 32], f16)
            nc.scalar.dma_start(spk_t[:], spk[:])

            ps = psum_pool.tile([32, TSH], f32)
            ot = out_pool.tile([32, TSH], f32)

            ngroups = SCHUNKS // GROUP
            for g in range(ngroups):
                at = adj_pool.tile([P, GROUP * TSH], f16)
                nc.sync.dma_start(at[:], a16[g * P : (g + 1) * P, :])
                last_group = g == ngroups - 1
                if not last_group:
                    for nl in range(GROUP):
                        n = g * GROUP + nl
                        w = spk_t[:, n * 32 : (n + 1) * 32]
                        for j in range(NJ):
                            c0 = nl * TSH + j * NFREE
                            nc.tensor.matmul(
                                ps[:, j * NFREE : (j + 1) * NFREE],
                                w,
                                at[:, c0 : c0 + NFREE],
                                start=(n == 0),
                                stop=False,
                            )
                else:
                    # Final group: finish one PSUM bank at a time so the
                    # PSUM->SBUF copy and the output DMA overlap the
                    # remaining banks' matmuls instead of serializing after
                    # the last one.
                    for j in range(NJ):
                        sl = slice(j * NFREE, (j + 1) * NFREE)
                        for nl in range(GROUP):
                            n = g * GROUP + nl
                            w = spk_t[:, n * 32 : (n + 1) * 32]
                            nc.tensor.matmul(
                                ps[:, sl],
                                w,
                                at[:, nl * TSH + j * NFREE : nl * TSH + (j + 1) * NFREE],
                                start=False,
                                stop=(nl == GROUP - 1),
                            )
                        nc.vector.tensor_copy(ot[:, sl], ps[:, sl])
                        nc.sync.dma_start(out[:, sl], ot[:, sl])

    nc.compile()
    return nc


def _split_hi_lo(x32):
    """Split fp32 array into (hi, lo) fp16 parts with x32 ~= hi + lo."""
    hi = x32.astype(np.float16)
    lo = (x32 - hi.astype(np.float32)).astype(np.float16)
    return hi, lo


def _prep_inputs(spikes, adjacency):
    flat = np.ascontiguousarray(np.asarray(spikes, dtype=np.float32).reshape(B, N))
    adj = np.asarray(adjacency, dtype=np.float32)

    flatT = np.ascontiguousarray(flat.T)  # [N, B]
    fhi, flo = _split_hi_lo(flatT)
    spk = np.empty((SCHUNKS, P, 32), np.float16)  # [n, p, 2*B]
    spk[:, :, :B] = fhi.reshape(SCHUNKS, P, B)
    spk[:, :, B:] = flo.reshape(SCHUNKS, P, B)
    spk = np.ascontiguousarray(spk.transpose(1, 0, 2)).reshape(P, SCHUNKS * 32)

    adjT = adj.T  # [source, target] view (strided)
    in_maps = []
    for i in range(NCORES):
        a16 = adjT[:, i * TSH : (i + 1) * TSH].astype(np.float16)  # [N, TSH]
        # Pack GROUP source-chunks per DRAM row (see kernel comment).
        a16 = np.ascontiguousarray(
            a16.reshape(N // (GROUP * P), GROUP, P, TSH).transpose(0, 2, 1, 3)
        ).reshape(N // GROUP, GROUP * TSH)
        in_maps.append({"a16": a16, "spk": spk})
    return in_maps


def _run(in_maps, **kwargs):
    from concourse.bass_utils import run_bass_kernel_spmd

    if "nc" not in _cache:
        _cache["nc"] = _build_nc()
    return run_bass_kernel_spmd(
        _cache["nc"], in_maps, core_ids=list(range(NCORES)), **kwargs
    )


def kernel(spikes, adjacency):
    in_maps = _prep_inputs(spikes, adjacency)
    res = _run(in_maps)
    outs = [r["o"] for r in res.results]
    # Fold hi-weight rows (0:16) + lo-weight rows (16:32), concat target shards.
    full = np.concatenate([o[:B] + o[B:] for o in outs], axis=1)  # [B, N]
    return np.ascontiguousarray(full.reshape(B, H, W), dtype=np.float32)


# revision 14
# speedup vs baseline: 1.1704x; 1.1704x over previous
# Trainium2 Bass kernel for nn_AxonalConnections (gnn_message_passing).
#
# Computes out[B, H, W] = (spikes.reshape(B, N) @ adjacency.T).reshape(B, H, W)
# with B=16, H=W=128, N=16384 on 8 NeuronCores.
#
# Strategy (pure tensor parallelism, no collectives):
#   - Shard adjacency row-wise (target dim) across 8 cores: core i owns
#     target columns [i*2048, (i+1)*2048) of the output.
#   - Host-side, transpose each shard to [source, target] layout so the
#     contraction dim (source) lands on SBUF partitions with unit-stride DMAs.
#   - The kernel is HBM-bandwidth bound, so minimize bytes: adjacency is
#     shipped as fp16 (values are ~N(0, 0.02^2), well inside fp16 range;
#     2^-11 relative representation error -> ~1e-4 output error). fp32
#     matmul would also stream 4x slower through the PE; fp16 streams at
#     full rate (1 column/cycle).
#   - Spikes (tiny) are split into fp16 hi + fp16 lo (exact to ~2^-22) and
#     packed as the stationary operand [spikes_hi | spikes_lo] (32 columns).
#     PSUM accumulates [32, 2048] fp32; rows 0-15 = hi terms, rows 16-31 =
#     lo terms; host folds them and concatenates the target shards.
#
# Per-core traffic: 64 MiB adjacency + 1 MiB spikes; single-queue HWDGE DMA
# sustains ~420 GB/s -> ~155 us steady state + ~25 us head/tail.

import numpy as np

B = 16
H = 128
W = 128
N = H * W            # 16384 source == target size
NCORES = 8
TSH = N // NCORES    # 2048 target columns per core
P = 128              # SBUF partitions / contraction tile
SCHUNKS = N // P     # 128 source chunks
GROUP = 4            # source chunks per DMA (GROUP * 0.5 MiB per transfer)
NFREE = 512          # matmul moving free dim (one PSUM bank of fp32)

_cache = {}


def _build_nc():
    import concourse.mybir as mybir
    import concourse.tile as tile
    from concourse import bacc

    nc = bacc.Bacc(
        "TRN2",
        target_bir_lowering=False,
        debug=False,
        num_devices=NCORES,
    )
    # a16: adjacency shard, transposed to [source, target] fp16, with two
    # source-chunks packed per DRAM row so every DMA descriptor moves a
    # contiguous 8 KiB run per partition. 8 KiB packets are the measured
    # sweet spot: 4 KiB packets pay ~12 ns/packet overhead (320 GB/s) and
    # 16 KiB packets pipeline worse per engine (322 GB/s); 8 KiB sustains
    # ~400+ GB/s.
    #   a16[g2*128 + p, half*TSH + t] = fp16(adj[t0 + t, (2*g2 + half)*128 + p])
    a16 = nc.dram_tensor(
        "a16", [N // 2, 2 * TSH], mybir.dt.float16, kind="ExternalInput"
    ).ap()
    # spk: stationary weights, packed [P, SCHUNKS*32] fp16 where
    #   spk[p, n*32 + b]      = fp16_hi(spikes[b, n*128 + p])
    #   spk[p, n*32 + 16 + b] = fp16_lo(spikes[b, n*128 + p])
    spk = nc.dram_tensor(
        "spk", [P, SCHUNKS * 32], mybir.dt.float16, kind="ExternalInput"
    ).ap()
    out = nc.dram_tensor("o", [32, TSH], mybir.dt.float32, kind="ExternalOutput").ap()

    f32 = mybir.dt.float32
    f16 = mybir.dt.float16
    NJ = TSH // NFREE  # 4 PSUM banks

    with tile.TileContext(nc) as tc:
        with (
            tc.tile_pool(name="adj", bufs=5) as adj_pool,
            tc.tile_pool(name="spkp", bufs=1) as spk_pool,
            tc.tile_pool(name="psum", bufs=1, space="PSUM") as psum_pool,
            tc.tile_pool(name="outp", bufs=1) as out_pool,
        ):
            # Stationary weights go on the ACT HWDGE ring so the transfer
            # overlaps the first adjacency groups on the SP ring (the
            # two-queue engine-utilization penalty only matters in steady
            # state; here the ACT ring drains in the first few us). The
            # adjacency stream itself stays on ONE queue: splitting it
            # across SP and ACT makes each SDMA engine alternate queues per
            # packet, costing ~15% utilization (measured 320 vs 422 GB/s).
            spk_t = spk_pool.tile([P, SCHUNKS * 32], f16)
            nc.scalar.dma_start(spk_t[:], spk[:])

            ps = psum_pool.tile([32, TSH], f32)
            ot = out_pool.tile([32, TSH], f32)

            ngroups = SCHUNKS // GROUP
            rows = (GROUP // 2) * P  # packed DRAM rows per group
            for g in range(ngroups):
                at = adj_pool.tile([P, GROUP * TSH], f16)
                nc.sync.dma_start(
                    at[:].rearrange("p (n t) -> p n t", n=GROUP // 2),
                    a16[g * rows : (g + 1) * rows, :].rearrange(
                        "(n p) t -> p n t", p=P
                    ),
                )
                last_group = g == ngroups - 1
                if not last_group:
                    for nl in range(GROUP):
                        n = g * GROUP + nl
                        w = spk_t[:, n * 32 : (n + 1) * 32]
                        for j in range(NJ):
                            c0 = nl * TSH + j * NFREE
                            nc.tensor.matmul(
                                ps[:, j * NFREE : (j + 1) * NFREE],
                                w,
                                at[:, c0 : c0 + NFREE],
                                start=(n == 0),
                                stop=False,
                            )
                else:
                    # Final group: finish one PSUM bank at a time so the
                    # PSUM->SBUF copy and the output DMA overlap the
                    # remaining banks' matmuls instead of serializing after
                    # the last one.
                    for j in range(NJ):
                        sl = slice(j * NFREE, (j + 1) * NFREE)
                        for nl in range(GROUP):
                            n = g * GROUP + nl
                            w = spk_t[:, n * 32 : (n + 1) * 32]
                            nc.tensor.matmul(
                                ps[:, sl],
                                w,
                                at[:, nl * TSH + j * NFREE : nl * TSH + (j + 1) * NFREE],
                                start=False,
                                stop=(nl == GROUP - 1),
                            )
                        nc.vector.tensor_copy(ot[:, sl], ps[:, sl])
                        nc.sync.dma_start(out[:, sl], ot[:, sl])

    nc.compile()
    return nc


def _split_hi_lo(x32):
    """Split fp32 array into (hi, lo) fp16 parts with x32 ~= hi + lo."""
    hi = x32.astype(np.float16)
    lo = (x32 - hi.astype(np.float32)).astype(np.float16)
    return hi, lo


def _prep_inputs(spikes, adjacency):
    flat = np.ascontiguousarray(np.asarray(spikes, dtype=np.float32).reshape(B, N))
    adj = np.asarray(adjacency, dtype=np.float32)

    flatT = np.ascontiguousarray(flat.T)  # [N, B]
    fhi, flo = _split_hi_lo(flatT)
    spk = np.empty((SCHUNKS, P, 32), np.float16)  # [n, p, 2*B]
    spk[:, :, :B] = fhi.reshape(SCHUNKS, P, B)
    spk[:, :, B:] = flo.reshape(SCHUNKS, P, B)
    spk = np.ascontiguousarray(spk.transpose(1, 0, 2)).reshape(P, SCHUNKS * 32)

    adjT = adj.T  # [source, target] view (strided)
    in_maps = []
    for i in range(NCORES):
        a16 = adjT[:, i * TSH : (i + 1) * TSH].astype(np.float16)  # [N, TSH]
        # Pack two source-chunks per DRAM row (see kernel comment).
        a16 = np.ascontiguousarray(
            a16.reshape(N // (2 * P), 2, P, TSH).transpose(0, 2, 1, 3)
        ).reshape(N // 2, 2 * TSH)
        in_maps.append({"a16": a16, "spk": spk})
    return in_maps


def _run(in_maps, **kwargs):
    from concourse.bass_utils import run_bass_kernel_spmd

    if "nc" not in _cache:
        _cache["nc"] = _build_nc()
    return run_bass_kernel_spmd(
        _cache["nc"], in_maps, core_ids=list(range(NCORES)), **kwargs
    )


def kernel(spikes, adjacency):
    in_maps = _prep_inputs(spikes, adjacency)
    res = _run(in_maps)
    outs = [r["o"] for r in res.results]
    # Fold hi-weight rows (0:16) + lo-weight rows (16:32), concat target shards.
    full = np.concatenate([o[:B] + o[B:] for o in outs], axis=1)  # [B, N]
    return np.ascontiguousarray(full.reshape(B, H, W), dtype=np.float32)


# revision 15
# speedup vs baseline: 4.3355x; 3.7042x over previous
# Trainium2 Bass kernel for nn_AxonalConnections (gnn_message_passing).
#
# Computes out[B, H, W] = (spikes.reshape(B, N) @ adjacency.T).reshape(B, H, W)
# with B=16, H=W=128, N=16384 on 8 NeuronCores.
#
# Strategy (pure tensor parallelism, no collectives):
#   - Shard adjacency row-wise (target dim) across 8 cores: core i owns
#     target columns [i*2048, (i+1)*2048) of the output.
#   - Host-side, transpose each shard to [source, target] layout so the
#     contraction dim (source) lands on SBUF partitions with unit-stride DMAs.
#   - The kernel is HBM/DMA-bandwidth bound, so minimize bytes:
#     * adjacency ships as fp16 (values ~N(0, 0.02^2), well inside fp16
#       range; 2^-11 relative representation error -> ~1e-4 output error).
#       fp32 would also stream 4x slower through the PE; fp16 streams at
#       full rate (1 column/cycle).
#     * input-adaptive source pruning: per shard, only source rows between
#       the first and last nonzero 128-row chunk contribute; the host
#       detects that range and ships only it. For conv-structured
#       adjacencies this is ~22/128 chunks; for dense inputs it degrades
#       to the full range and stays correct.
#   - Spikes (tiny) are split into fp16 hi + fp16 lo (exact to ~2^-22) and
#     packed as the stationary operand [spikes_hi | spikes_lo] (32 columns).
#     PSUM accumulates [32, 2048] fp32; rows 0-15 = hi terms, rows 16-31 =
#     lo terms; host folds them and concatenates the target shards.
#
# Single-queue HWDGE DMA with 8 KiB per-partition runs sustains ~410 GB/s
# (95% of the 435 GB/s SBUF-AXI fabric ceiling).

import numpy as np

B = 16
H = 128
W = 128
N = H * W            # 16384 source == target size
NCORES = 8
TSH = N // NCORES    # 2048 target columns per core
P = 128              # SBUF partitions / contraction tile
SCHUNKS = N // P     # 128 source chunks
GROUP = 4            # source chunks per DMA (GROUP * 0.5 MiB per transfer)
NFREE = 512          # matmul moving free dim (one PSUM bank of fp32)

_cache = {}


def _build_nc(n_chunks):
    """Build + compile the SPMD Bass program for `n_chunks` source chunks."""
    import concourse.mybir as mybir
    import concourse.tile as tile
    from concourse import bacc

    assert n_chunks % GROUP == 0 and n_chunks > 0

    nc = bacc.Bacc(
        "TRN2",
        target_bir_lowering=False,
        debug=False,
        num_devices=NCORES,
    )
    # a16: live slice of the transposed fp16 adjacency shard, with two
    # source-chunks packed per DRAM row so every DMA descriptor moves a
    # contiguous 8 KiB run per partition. 8 KiB packets are the measured
    # DMA sweet spot: 4 KiB packets pay ~12 ns/packet overhead (320 GB/s)
    # and 16 KiB packets pipeline worse per engine (322 GB/s); 8 KiB
    # sustains ~410 GB/s.
    #   a16[g2*128 + p, half*TSH + t] =
    #       fp16(adj[t0 + t, (c_lo + 2*g2 + half)*128 + p])
    a16 = nc.dram_tensor(
        "a16", [n_chunks * P // 2, 2 * TSH], mybir.dt.float16, kind="ExternalInput"
    ).ap()
    # spk: stationary weights for the live chunk range, packed
    # [P, n_chunks*32] fp16 where
    #   spk[p, k*32 + b]      = fp16_hi(spikes[b, (c_lo + k)*128 + p])
    #   spk[p, k*32 + 16 + b] = fp16_lo(spikes[b, (c_lo + k)*128 + p])
    spk = nc.dram_tensor(
        "spk", [P, n_chunks * 32], mybir.dt.float16, kind="ExternalInput"
    ).ap()
    out = nc.dram_tensor("o", [32, TSH], mybir.dt.float32, kind="ExternalOutput").ap()

    f32 = mybir.dt.float32
    f16 = mybir.dt.float16
    NJ = TSH // NFREE  # 4 PSUM banks

    with tile.TileContext(nc) as tc:
        with (
            tc.tile_pool(name="adj", bufs=min(6, n_chunks // GROUP)) as adj_pool,
            tc.tile_pool(name="spkp", bufs=1) as spk_pool,
            tc.tile_pool(name="psum", bufs=1, space="PSUM") as psum_pool,
            tc.tile_pool(name="outp", bufs=1) as out_pool,
        ):
            # Stationary weights go on the ACT HWDGE ring so the transfer
            # overlaps the first adjacency groups on the SP ring. The
            # adjacency stream itself stays on ONE queue: splitting it
            # across SP and ACT makes each SDMA engine alternate queues per
            # packet, costing ~15% utilization (measured 320 vs 422 GB/s).
            spk_t = spk_pool.tile([P, n_chunks * 32], f16)
            nc.scalar.dma_start(spk_t[:], spk[:])

            ps = psum_pool.tile([32, TSH], f32)
            ot = out_pool.tile([32, TSH], f32)

            ngroups = n_chunks // GROUP
            rows = (GROUP // 2) * P  # packed DRAM rows per group
            for g in range(ngroups):
                at = adj_pool.tile([P, GROUP * TSH], f16)
                nc.sync.dma_start(
                    at[:].rearrange("p (n t) -> p n t", n=GROUP // 2),
                    a16[g * rows : (g + 1) * rows, :].rearrange(
                        "(n p) t -> p n t", p=P
                    ),
                )
                last_group = g == ngroups - 1
                if not last_group:
                    for nl in range(GROUP):
                        n = g * GROUP + nl
                        w = spk_t[:, n * 32 : (n + 1) * 32]
                        for j in range(NJ):
                            c0 = nl * TSH + j * NFREE
                            nc.tensor.matmul(
                                ps[:, j * NFREE : (j + 1) * NFREE],
                                w,
                                at[:, c0 : c0 + NFREE],
                                start=(n == 0),
                                stop=False,
                            )
                else:
                    # Final group: finish one PSUM bank at a time so the
                    # PSUM->SBUF copy and the output DMA overlap the
                    # remaining banks' matmuls instead of serializing after
                    # the last one.
                    for j in range(NJ):
                        sl = slice(j * NFREE, (j + 1) * NFREE)
                        for nl in range(GROUP):
                            n = g * GROUP + nl
                            w = spk_t[:, n * 32 : (n + 1) * 32]
                            nc.tensor.matmul(
                                ps[:, sl],
                                w,
                                at[:, nl * TSH + j * NFREE : nl * TSH + (j + 1) * NFREE],
                                start=(n == 0),
                                stop=(nl == GROUP - 1),
                            )
                        nc.vector.tensor_copy(ot[:, sl], ps[:, sl])
                        nc.sync.dma_start(out[:, sl], ot[:, sl])

    nc.compile()
    return nc


def _get_nc(n_chunks):
    key = ("nc", n_chunks)
    if key not in _cache:
        _cache[key] = _build_nc(n_chunks)
    return _cache[key]


def _split_hi_lo(x32):
    """Split fp32 array into (hi, lo) fp16 parts with x32 ~= hi + lo."""
    hi = x32.astype(np.float16)
    lo = (x32 - hi.astype(np.float32)).astype(np.float16)
    return hi, lo


def _prep_inputs(spikes, adjacency):
    flat = np.ascontiguousarray(np.asarray(spikes, dtype=np.float32).reshape(B, N))
    adj = np.asarray(adjacency, dtype=np.float32)

    # Live source-chunk range per target shard: chunk c contributes to core
    # i's outputs only if adj[i*TSH:(i+1)*TSH, c*128:(c+1)*128] has any
    # nonzero. Shipping [first_live, last_live] keeps the kernel exact for
    # every input while skipping the all-zero bands of conv-structured
    # adjacencies.
    blocks = np.any(
        adj.reshape(NCORES, TSH, SCHUNKS, P) != 0.0, axis=(1, 3)
    )  # [NCORES, SCHUNKS]
    c_lo = np.zeros(NCORES, np.int64)
    c_len = np.full(NCORES, SCHUNKS, np.int64)
    for i in range(NCORES):
        nz = np.nonzero(blocks[i])[0]
        if len(nz):
            c_lo[i], c_len[i] = nz[0], nz[-1] - nz[0] + 1
        else:
            c_lo[i], c_len[i] = 0, 1
    n_chunks = int(min(SCHUNKS, -(-int(c_len.max()) // GROUP) * GROUP))
    # Clamp each core's range to [0, SCHUNKS - n_chunks].
    c_lo = np.minimum(c_lo, SCHUNKS - n_chunks)

    flatT = np.ascontiguousarray(flat.T)  # [N, B]
    fhi, flo = _split_hi_lo(flatT)
    spk_full = np.empty((SCHUNKS, P, 32), np.float16)  # [c, p, 2*B]
    spk_full[:, :, :B] = fhi.reshape(SCHUNKS, P, B)
    spk_full[:, :, B:] = flo.reshape(SCHUNKS, P, B)

    adjT = adj.T  # [source, target] view (strided)
    in_maps = []
    for i in range(NCORES):
        lo = int(c_lo[i])
        s0, s1 = lo * P, (lo + n_chunks) * P
        a16 = adjT[s0:s1, i * TSH : (i + 1) * TSH].astype(np.float16)
        # Pack two source-chunks per DRAM row (see kernel comment).
        a16 = np.ascontiguousarray(
            a16.reshape(n_chunks // 2, 2, P, TSH).transpose(0, 2, 1, 3)
        ).reshape(n_chunks * P // 2, 2 * TSH)
        spk = np.ascontiguousarray(
            spk_full[lo : lo + n_chunks].transpose(1, 0, 2)
        ).reshape(P, n_chunks * 32)
        in_maps.append({"a16": a16, "spk": spk})
    return n_chunks, in_maps


def _run(n_chunks, in_maps, **kwargs):
    from concourse.bass_utils import run_bass_kernel_spmd

    return run_bass_kernel_spmd(
        _get_nc(n_chunks), in_maps, core_ids=list(range(NCORES)), **kwargs
    )


def kernel(spikes, adjacency):
    n_chunks, in_maps = _prep_inputs(spikes, adjacency)
    res = _run(n_chunks, in_maps)
    outs = [r["o"] for r in res.results]
    # Fold hi-weight rows (0:16) + lo-weight rows (16:32), concat target shards.
    full = np.concatenate([o[:B] + o[B:] for o in outs], axis=1)  # [B, N]
    return np.ascontiguousarray(full.reshape(B, H, W), dtype=np.float32)


# revision 19
# speedup vs baseline: 4.4044x; 1.0159x over previous
# Trainium2 Bass kernel for nn_AxonalConnections (gnn_message_passing).
#
# Computes out[B, H, W] = (spikes.reshape(B, N) @ adjacency.T).reshape(B, H, W)
# with B=16, H=W=128, N=16384 on 8 NeuronCores.
#
# Strategy (pure tensor parallelism, no collectives):
#   - Shard adjacency row-wise (target dim) across 8 cores: core i owns
#     target columns [i*2048, (i+1)*2048) of the output.
#   - Host-side, transpose each shard to [source, target] layout so the
#     contraction dim (source) lands on SBUF partitions with unit-stride DMAs.
#   - The kernel is HBM/DMA-bandwidth bound, so minimize bytes:
#     * adjacency ships as fp16 (values ~N(0, 0.02^2), well inside fp16
#       range; 2^-11 relative representation error -> ~1e-4 output error).
#       fp32 would also stream 4x slower through the PE; fp16 streams at
#       full rate (1 column/cycle).
#     * input-adaptive source pruning: per shard, only source rows between
#       the first and last nonzero 128-row chunk contribute; the host
#       detects that range and ships only it. For conv-structured
#       adjacencies this is ~22/128 chunks; for dense inputs it degrades
#       to the full range and stays correct.
#   - Spikes (tiny) are split into fp16 hi + fp16 lo (exact to ~2^-22) and
#     packed as the stationary operand [spikes_hi | spikes_lo] (32 columns).
#     PSUM accumulates [32, 2048] fp32; rows 0-15 = hi terms, rows 16-31 =
#     lo terms; host folds them and concatenates the target shards.
#
# Single-queue HWDGE DMA with 8 KiB per-partition runs sustains ~410 GB/s
# (95% of the 435 GB/s SBUF-AXI fabric ceiling).

import numpy as np

B = 16
H = 128
W = 128
N = H * W            # 16384 source == target size
NCORES = 8
TSH = N // NCORES    # 2048 target columns per core
P = 128              # SBUF partitions / contraction tile
SCHUNKS = N // P     # 128 source chunks
GROUP = 2            # source chunks per DMA (GROUP * 0.5 MiB per transfer)
NFREE = 512          # matmul moving free dim (one PSUM bank of fp32)

_cache = {}


def _build_nc(n_chunks):
    """Build + compile the SPMD Bass program for `n_chunks` source chunks."""
    import concourse.mybir as mybir
    import concourse.tile as tile
    from concourse import bacc

    assert n_chunks % GROUP == 0 and n_chunks > 0

    nc = bacc.Bacc(
        "TRN2",
        target_bir_lowering=False,
        debug=False,
        num_devices=NCORES,
    )
    # a16: live slice of the transposed fp16 adjacency shard, with two
    # source-chunks packed per DRAM row so every DMA descriptor moves a
    # contiguous 8 KiB run per partition. 8 KiB packets are the measured
    # DMA sweet spot: 4 KiB packets pay ~12 ns/packet overhead (320 GB/s)
    # and 16 KiB packets pipeline worse per engine (322 GB/s); 8 KiB
    # sustains ~410 GB/s.
    #   a16[g2*128 + p, half*TSH + t] =
    #       fp16(adj[t0 + t, (c_lo + 2*g2 + half)*128 + p])
    a16 = nc.dram_tensor(
        "a16", [n_chunks * P // 2, 2 * TSH], mybir.dt.float16, kind="ExternalInput"
    ).ap()
    # spk: stationary weights for the live chunk range, packed
    # [P, n_chunks*32] fp16 where
    #   spk[p, k*32 + b]      = fp16_hi(spikes[b, (c_lo + k)*128 + p])
    #   spk[p, k*32 + 16 + b] = fp16_lo(spikes[b, (c_lo + k)*128 + p])
    spk = nc.dram_tensor(
        "spk", [P, n_chunks * 32], mybir.dt.float16, kind="ExternalInput"
    ).ap()
    out = nc.dram_tensor("o", [32, TSH], mybir.dt.float32, kind="ExternalOutput").ap()

    f32 = mybir.dt.float32
    f16 = mybir.dt.float16
    NJ = TSH // NFREE  # 4 PSUM banks

    with tile.TileContext(nc) as tc:
        with (
            tc.tile_pool(name="adj", bufs=min(8, n_chunks // GROUP)) as adj_pool,
            tc.tile_pool(name="spkp", bufs=1) as spk_pool,
            tc.tile_pool(name="psum", bufs=1, space="PSUM") as psum_pool,
            tc.tile_pool(name="outp", bufs=1) as out_pool,
        ):
            # Stationary weights go on the ACT HWDGE ring so the transfer
            # overlaps the first adjacency groups on the SP ring. The
            # adjacency stream itself stays on ONE queue: splitting it
            # across SP and ACT makes each SDMA engine alternate queues per
            # packet, costing ~15% utilization (measured 320 vs 422 GB/s).
            spk_t = spk_pool.tile([P, n_chunks * 32], f16)
            nc.scalar.dma_start(spk_t[:], spk[:])

            # One PSUM tile per output bank so the tail copies don't
            # serialize against the other banks' matmuls (Tile tracks
            # PSUM dependencies at tile granularity).
            ps = [
                psum_pool.tile([32, NFREE], f32, name=f"ps{j}", tag=f"ps{j}")
                for j in range(NJ)
            ]
            ot = out_pool.tile([32, TSH], f32)

            ngroups = n_chunks // GROUP
            rows = (GROUP // 2) * P  # packed DRAM rows per group
            for g in range(ngroups):
                at = adj_pool.tile([P, GROUP * TSH], f16)
                nc.sync.dma_start(
                    at[:].rearrange("p (n t) -> p n t", n=GROUP // 2),
                    a16[g * rows : (g + 1) * rows, :].rearrange(
                        "(n p) t -> p n t", p=P
                    ),
                )
                last_group = g == ngroups - 1
                if not last_group:
                    for nl in range(GROUP):
                        n = g * GROUP + nl
                        w = spk_t[:, n * 32 : (n + 1) * 32]
                        for j in range(NJ):
                            c0 = nl * TSH + j * NFREE
                            nc.tensor.matmul(
                                ps[j][:, :],
                                w,
                                at[:, c0 : c0 + NFREE],
                                start=(n == 0),
                                stop=False,
                            )
                else:
                    # Final group: finish one PSUM bank at a time so the
                    # PSUM->SBUF copy and the output DMA overlap the
                    # remaining banks' matmuls instead of serializing after
                    # the last one.
                    for j in range(NJ):
                        sl = slice(j * NFREE, (j + 1) * NFREE)
                        for nl in range(GROUP):
                            n = g * GROUP + nl
                            w = spk_t[:, n * 32 : (n + 1) * 32]
                            nc.tensor.matmul(
                                ps[j][:, :],
                                w,
                                at[:, nl * TSH + j * NFREE : nl * TSH + (j + 1) * NFREE],
                                start=(n == 0),
                                stop=(nl == GROUP - 1),
                            )
                        nc.vector.tensor_copy(ot[:, sl], ps[j][:, :])
                        nc.sync.dma_start(out[:, sl], ot[:, sl])

    nc.compile()
    return nc


def _get_nc(n_chunks):
    key = ("nc", n_chunks)
    if key not in _cache:
        _cache[key] = _build_nc(n_chunks)
    return _cache[key]


def _split_hi_lo(x32):
    """Split fp32 array into (hi, lo) fp16 parts with x32 ~= hi + lo."""
    hi = x32.astype(np.float16)
    lo = (x32 - hi.astype(np.float32)).astype(np.float16)
    return hi, lo


def _prep_inputs(spikes, adjacency):
    flat = np.ascontiguousarray(np.asarray(spikes, dtype=np.float32).reshape(B, N))
    adj = np.asarray(adjacency, dtype=np.float32)

    # Live source-chunk range per target shard: chunk c contributes to core
    # i's outputs only if adj[i*TSH:(i+1)*TSH, c*128:(c+1)*128] has any
    # nonzero. Shipping [first_live, last_live] keeps the kernel exact for
    # every input while skipping the all-zero bands of conv-structured
    # adjacencies.
    blocks = np.any(
        adj.reshape(NCORES, TSH, SCHUNKS, P) != 0.0, axis=(1, 3)
    )  # [NCORES, SCHUNKS]
    c_lo = np.zeros(NCORES, np.int64)
    c_len = np.full(NCORES, SCHUNKS, np.int64)
    for i in range(NCORES):
        nz = np.nonzero(blocks[i])[0]
        if len(nz):
            c_lo[i], c_len[i] = nz[0], nz[-1] - nz[0] + 1
        else:
            c_lo[i], c_len[i] = 0, 1
    n_chunks = int(min(SCHUNKS, -(-int(c_len.max()) // GROUP) * GROUP))
    # Clamp each core's range to [0, SCHUNKS - n_chunks].
    c_lo = np.minimum(c_lo, SCHUNKS - n_chunks)

    flatT = np.ascontiguousarray(flat.T)  # [N, B]
    fhi, flo = _split_hi_lo(flatT)
    spk_full = np.empty((SCHUNKS, P, 32), np.float16)  # [c, p, 2*B]
    spk_full[:, :, :B] = fhi.reshape(SCHUNKS, P, B)
    spk_full[:, :, B:] = flo.reshape(SCHUNKS, P, B)

    adjT = adj.T  # [source, target] view (strided)
    in_maps = []
    for i in range(NCORES):
        lo = int(c_lo[i])
        s0, s1 = lo * P, (lo + n_chunks) * P
        a16 = adjT[s0:s1, i * TSH : (i + 1) * TSH].astype(np.float16)
        # Pack two source-chunks per DRAM row (see kernel comment).
        a16 = np.ascontiguousarray(
            a16.reshape(n_chunks // 2, 2, P, TSH).transpose(0, 2, 1, 3)
        ).reshape(n_chunks * P // 2, 2 * TSH)
        spk = np.ascontiguousarray(
            spk_full[lo : lo + n_chunks].transpose(1, 0, 2)
        ).reshape(P, n_chunks * 32)
        in_maps.append({"a16": a16, "spk": spk})
    return n_chunks, in_maps


def _run(n_chunks, in_maps, **kwargs):
    from concourse.bass_utils import run_bass_kernel_spmd

    return run_bass_kernel_spmd(
        _get_nc(n_chunks), in_maps, core_ids=list(range(NCORES)), **kwargs
    )


def kernel(spikes, adjacency):
    n_chunks, in_maps = _prep_inputs(spikes, adjacency)
    res = _run(n_chunks, in_maps)
    outs = [r["o"] for r in res.results]
    # Fold hi-weight rows (0:16) + lo-weight rows (16:32), concat target shards.
    full = np.concatenate([o[:B] + o[B:] for o in outs], axis=1)  # [B, N]
    return np.ascontiguousarray(full.reshape(B, H, W), dtype=np.float32)


# revision 20
# speedup vs baseline: 4.4056x; 1.0003x over previous
# Trainium2 Bass kernel for nn_AxonalConnections (gnn_message_passing).
#
# Computes out[B, H, W] = (spikes.reshape(B, N) @ adjacency.T).reshape(B, H, W)
# with B=16, H=W=128, N=16384 on 8 NeuronCores.
#
# Strategy (pure tensor parallelism, no collectives):
#   - Shard adjacency row-wise (target dim) across 8 cores: core i owns
#     target columns [i*2048, (i+1)*2048) of the output.
#   - Host-side, transpose each shard to [source, target] layout so the
#     contraction dim (source) lands on SBUF partitions with unit-stride DMAs.
#   - The kernel is HBM/DMA-bandwidth bound, so minimize bytes:
#     * adjacency ships as fp16 (values ~N(0, 0.02^2), well inside fp16
#       range; 2^-11 relative representation error -> ~1e-4 output error).
#       fp32 would also stream 4x slower through the PE; fp16 streams at
#       full rate (1 column/cycle).
#     * input-adaptive source pruning: per shard, only source rows between
#       the first and last nonzero 128-row chunk contribute; the host
#       detects that range and ships only it. For conv-structured
#       adjacencies this is ~22/128 chunks; for dense inputs it degrades
#       to the full range and stays correct.
#   - Spikes (tiny) are split into fp16 hi + fp16 lo (exact to ~2^-22) and
#     packed as the stationary operand [spikes_hi | spikes_lo] (32 columns).
#     PSUM accumulates [32, 2048] fp32; rows 0-15 = hi terms, rows 16-31 =
#     lo terms; host folds them and concatenates the target shards.
#
# Single-queue HWDGE DMA with 8 KiB per-partition runs sustains ~410 GB/s
# (95% of the 435 GB/s SBUF-AXI fabric ceiling).

import numpy as np

B = 16
H = 128
W = 128
N = H * W            # 16384 source == target size
NCORES = 8
TSH = N // NCORES    # 2048 target columns per core
P = 128              # SBUF partitions / contraction tile
SCHUNKS = N // P     # 128 source chunks
GROUP = 2            # source chunks per DMA (GROUP * 0.5 MiB per transfer)
NFREE = 512          # matmul moving free dim (one PSUM bank of fp32)

_cache = {}


def _build_nc(n_chunks):
    """Build + compile the SPMD Bass program for `n_chunks` source chunks."""
    import concourse.mybir as mybir
    import concourse.tile as tile
    from concourse import bacc

    assert n_chunks % GROUP == 0 and n_chunks > 0

    nc = bacc.Bacc(
        "TRN2",
        target_bir_lowering=False,
        debug=False,
        num_devices=NCORES,
    )
    # a16: live slice of the transposed fp16 adjacency shard, with two
    # source-chunks packed per DRAM row so every DMA descriptor moves a
    # contiguous 8 KiB run per partition. 8 KiB packets are the measured
    # DMA sweet spot: 4 KiB packets pay ~12 ns/packet overhead (320 GB/s)
    # and 16 KiB packets pipeline worse per engine (322 GB/s); 8 KiB
    # sustains ~410 GB/s.
    #   a16[g2*128 + p, half*TSH + t] =
    #       fp16(adj[t0 + t, (c_lo + 2*g2 + half)*128 + p])
    a16 = nc.dram_tensor(
        "a16", [n_chunks * P // 2, 2 * TSH], mybir.dt.float16, kind="ExternalInput"
    ).ap()
    # spk: stationary weights for the live chunk range, packed
    # [P, n_chunks*32] fp16 where
    #   spk[p, k*32 + b]      = fp16_hi(spikes[b, (c_lo + k)*128 + p])
    #   spk[p, k*32 + 16 + b] = fp16_lo(spikes[b, (c_lo + k)*128 + p])
    spk = nc.dram_tensor(
        "spk", [P, n_chunks * 32], mybir.dt.float16, kind="ExternalInput"
    ).ap()
    out = nc.dram_tensor("o", [32, TSH], mybir.dt.float32, kind="ExternalOutput").ap()

    f32 = mybir.dt.float32
    f16 = mybir.dt.float16
    NJ = TSH // NFREE  # 4 PSUM banks

    with tile.TileContext(nc) as tc:
        with (
            # Enough buffers to prefetch the whole live stream when it is
            # small (sparse inputs): DMA issue then never gates on PE
            # progress. Capped so the dense fallback still fits SBUF.
            tc.tile_pool(name="adj", bufs=min(16, n_chunks // GROUP)) as adj_pool,
            tc.tile_pool(name="spkp", bufs=1) as spk_pool,
            tc.tile_pool(name="psum", bufs=1, space="PSUM") as psum_pool,
            tc.tile_pool(name="outp", bufs=1) as out_pool,
        ):
            # Stationary weights go on the ACT HWDGE ring so the transfer
            # overlaps the first adjacency groups on the SP ring. The
            # adjacency stream itself stays on ONE queue: splitting it
            # across SP and ACT makes each SDMA engine alternate queues per
            # packet, costing ~15% utilization (measured 320 vs 422 GB/s).
            spk_t = spk_pool.tile([P, n_chunks * 32], f16)
            nc.scalar.dma_start(spk_t[:], spk[:])

            # One PSUM tile per output bank so the tail copies don't
            # serialize against the other banks' matmuls (Tile tracks
            # PSUM dependencies at tile granularity).
            ps = [
                psum_pool.tile([32, NFREE], f32, name=f"ps{j}", tag=f"ps{j}")
                for j in range(NJ)
            ]
            ot = out_pool.tile([32, TSH], f32)

            ngroups = n_chunks // GROUP
            rows = (GROUP // 2) * P  # packed DRAM rows per group
            for g in range(ngroups):
                at = adj_pool.tile([P, GROUP * TSH], f16)
                nc.sync.dma_start(
                    at[:].rearrange("p (n t) -> p n t", n=GROUP // 2),
                    a16[g * rows : (g + 1) * rows, :].rearrange(
                        "(n p) t -> p n t", p=P
                    ),
                )
                last_group = g == ngroups - 1
                if not last_group:
                    for nl in range(GROUP):
                        n = g * GROUP + nl
                        w = spk_t[:, n * 32 : (n + 1) * 32]
                        for j in range(NJ):
                            c0 = nl * TSH + j * NFREE
                            nc.tensor.matmul(
                                ps[j][:, :],
                                w,
                                at[:, c0 : c0 + NFREE],
                                start=(n == 0),
                                stop=False,
                            )
                else:
                    # Final group: finish one PSUM bank at a time so the
                    # PSUM->SBUF copy and the output DMA overlap the
                    # remaining banks' matmuls instead of serializing after
                    # the last one.
                    for j in range(NJ):
                        sl = slice(j * NFREE, (j + 1) * NFREE)
                        for nl in range(GROUP):
                            n = g * GROUP + nl
                            w = spk_t[:, n * 32 : (n + 1) * 32]
                            nc.tensor.matmul(
                                ps[j][:, :],
                                w,
                                at[:, nl * TSH + j * NFREE : nl * TSH + (j + 1) * NFREE],
                                start=(n == 0),
                                stop=(nl == GROUP - 1),
                            )
                        nc.vector.tensor_copy(ot[:, sl], ps[j][:, :])
                        nc.sync.dma_start(out[:, sl], ot[:, sl])

    nc.compile()
    return nc


def _get_nc(n_chunks):
    key = ("nc", n_chunks)
    if key not in _cache:
        _cache[key] = _build_nc(n_chunks)
    return _cache[key]


def _split_hi_lo(x32):
    """Split fp32 array into (hi, lo) fp16 parts with x32 ~= hi + lo."""
    hi = x32.astype(np.float16)
    lo = (x32 - hi.astype(np.float32)).astype(np.float16)
    return hi, lo


def _prep_inputs(spikes, adjacency):
    flat = np.ascontiguousarray(np.asarray(spikes, dtype=np.float32).reshape(B, N))
    adj = np.asarray(adjacency, dtype=np.float32)

    # Live source-chunk range per target shard: chunk c contributes to core
    # i's outputs only if adj[i*TSH:(i+1)*TSH, c*128:(c+1)*128] has any
    # nonzero. Shipping [first_live, last_live] keeps the kernel exact for
    # every input while skipping the all-zero bands of conv-structured
    # adjacencies.
    blocks = np.any(
        adj.reshape(NCORES, TSH, SCHUNKS, P) != 0.0, axis=(1, 3)
    )  # [NCORES, SCHUNKS]
    c_lo = np.zeros(NCORES, np.int64)
    c_len = np.full(NCORES, SCHUNKS, np.int64)
    for i in range(NCORES):
        nz = np.nonzero(blocks[i])[0]
        if len(nz):
            c_lo[i], c_len[i] = nz[0], nz[-1] - nz[0] + 1
        else:
            c_lo[i], c_len[i] = 0, 1
    n_chunks = int(min(SCHUNKS, -(-int(c_len.max()) // GROUP) * GROUP))
    # Clamp each core's range to [0, SCHUNKS - n_chunks].
    c_lo = np.minimum(c_lo, SCHUNKS - n_chunks)

    flatT = np.ascontiguousarray(flat.T)  # [N, B]
    fhi, flo = _split_hi_lo(flatT)
    spk_full = np.empty((SCHUNKS, P, 32), np.float16)  # [c, p, 2*B]
    spk_full[:, :, :B] = fhi.reshape(SCHUNKS, P, B)
    spk_full[:, :, B:] = flo.reshape(SCHUNKS, P, B)

    adjT = adj.T  # [source, target] view (strided)
    in_maps = []
    for i in range(NCORES):
        lo = int(c_lo[i])
        s0, s1 = lo * P, (lo + n_chunks) * P
        a16 = adjT[s0:s1, i * TSH : (i + 1) * TSH].astype(np.float16)
        # Pack two source-chunks per DRAM row (see kernel comment).
        a16 = np.ascontiguousarray(
            a16.reshape(n_chunks // 2, 2, P, TSH).transpose(0, 2, 1, 3)
        ).reshape(n_chunks * P // 2, 2 * TSH)
        spk = np.ascontiguousarray(
            spk_full[lo : lo + n_chunks].transpose(1, 0, 2)
        ).reshape(P, n_chunks * 32)
        in_maps.append({"a16": a16, "spk": spk})
    return n_chunks, in_maps


def _run(n_chunks, in_maps, **kwargs):
    from concourse.bass_utils import run_bass_kernel_spmd

    return run_bass_kernel_spmd(
        _get_nc(n_chunks), in_maps, core_ids=list(range(NCORES)), **kwargs
    )


def kernel(spikes, adjacency):
    n_chunks, in_maps = _prep_inputs(spikes, adjacency)
    res = _run(n_chunks, in_maps)
    outs = [r["o"] for r in res.results]
    # Fold hi-weight rows (0:16) + lo-weight rows (16:32), concat target shards.
    full = np.concatenate([o[:B] + o[B:] for o in outs], axis=1)  # [B, N]
    return np.ascontiguousarray(full.reshape(B, H, W), dtype=np.float32)
